# revision 1
# baseline (speedup 1.0000x reference)
"""GAT (PyG GATConv + Linear) on 8 Trainium2 NeuronCores — fp16 pipeline.

v2 changes vs baseline:
  - All per-edge streams in fp16 (halves the dominant SF DMA: 41.7 -> 20.8 MB/core).
  - Feature-major tile layout [P, t, f, j] so every DVE op has a dense step-1
    inner axis (enables 16-bit packed modes); the segment reduce's inner axis
    is now dense (was strided by 24 in the baseline).
  - leaky-relu and exp both on the scalar (ACT) engine, freeing the DVE.
  - F / AD / OUT use partition-major dense HBM layouts (128 large descriptors
    instead of 12800 tiny ones); host does the rank un-permutation.
"""
import os
import sys
import time

for _p in ("/opt/trn_rl_repo", "/root/.axon_site/_ro/trn_rl_repo"):
    if os.path.isdir(_p) and _p not in sys.path:
        sys.path.append(_p)

import numpy as np
import ml_dtypes

STREAM_DT = "bf16"            # "bf16" | "f16"  (per-edge stream + work dtype)
NP_DT = {"bf16": ml_dtypes.bfloat16, "f16": np.float16}[STREAM_DT]

N_NODES = 100000
N_CORES = 8
IN_F = 128
HEADS = 8
OUT_C = 2
HC = HEADS * OUT_C          # 16
NEG_SLOPE = 0.2
NODES_PER_CORE = N_NODES // N_CORES   # 12500
P = 128
NT = 100                              # tiles
GRP = 5                               # compute group (tiles per op chain)
SGT = 20                              # tiles per DMA super-group
NP = NT * P                           # 12800
PAD_ASRC = -30000.0                   # fp16-safe: lrelu*exp underflows to 0
# feature order inside SF rows: [q-ch0 h0..h7 | q-ch1 h0..h7 | a_src h0..h7]
# (channel-major so the p*h mult is two dense 4-D tensor_tensor ops)
PERM16 = [h * OUT_C + c for c in range(OUT_C) for h in range(HEADS)]


# ----------------------------------------------------------------- host prep
def _build_shards(edge_index):
    src = np.asarray(edge_index[0], dtype=np.int64)
    dst = np.asarray(edge_index[1], dtype=np.int64)
    loops = np.arange(N_NODES, dtype=np.int64)
    src = np.concatenate([src, loops])
    dst = np.concatenate([dst, loops])

    core = dst // NODES_PER_CORE
    deg = np.bincount(dst, minlength=N_NODES)

    perms, srcs_by_core, dloc_by_core = [], [], []
    ptab_per_core = np.zeros((N_CORES, NT), np.int64)
    for c in range(N_CORES):
        lo = c * NODES_PER_CORE
        d = deg[lo:lo + NODES_PER_CORE]
        order = np.argsort(-d, kind="stable")
        perm = np.full(NP, -1, np.int64)
        perm[:NODES_PER_CORE] = np.arange(lo, lo + NODES_PER_CORE)[order]
        perms.append(perm)
        dd_pad = np.concatenate([d[order], np.zeros(NP - NODES_PER_CORE, np.int64)])
        ptab_per_core[c] = dd_pad.reshape(NT, P).max(axis=1)
        rank_of_node = np.empty(NODES_PER_CORE, np.int64)
        rank_of_node[order] = np.arange(NODES_PER_CORE)
        m = core == c
        srcs_by_core.append(src[m])
        dloc_by_core.append(rank_of_node[dst[m] - lo])

    ptab = np.maximum(ptab_per_core.max(axis=0), 1)
    ptab = np.repeat(ptab.reshape(NT // SGT, SGT).max(axis=1), SGT)
    ptab = ((ptab + 3) // 4) * 4      # two clean pairwise halvings per tile
    S = int((ptab * P).sum())
    tilebase = np.concatenate([[0], np.cumsum(ptab * P)[:-1]])

    slot_srcs = []
    for c in range(N_CORES):
        s = np.full(S, -1, np.int64)
        dloc = dloc_by_core[c]
        esrc = srcs_by_core[c]
        order = np.argsort(dloc, kind="stable")
        dloc_s = dloc[order]
        esrc_s = esrc[order]
        _, cnt = np.unique(dloc_s, return_counts=True)
        j = np.arange(len(dloc_s)) - np.repeat(np.cumsum(cnt) - cnt, cnt)
        ts = dloc_s // P
        ps = dloc_s % P
        s[tilebase[ts] + ps * ptab[ts] + j] = esrc_s
        slot_srcs.append(s)

    return {"perms": perms, "ptab": ptab, "tilebase": tilebase, "S": S,
            "slot_srcs": slot_srcs}


# ------------------------------------------------------------- bass kernels
def _build_kernel1(body_reps=1):
    import concourse.bacc as bacc
    import concourse.tile as tile
    import concourse.mybir as mybir

    f16 = {"bf16": mybir.dt.bfloat16, "f16": mybir.dt.float16}[STREAM_DT]
    nc = bacc.Bacc("TRN2", target_bir_lowering=False, debug=False,
                   enable_asserts=True, num_devices=N_CORES)
    xT = nc.dram_tensor("xT", [P, NP], f16, kind="ExternalInput").ap()
    Wt = nc.dram_tensor("Wt", [P, HC], f16, kind="ExternalInput").ap()
    asr = nc.dram_tensor("asr", [P, HC], f16, kind="ExternalInput").ap()
    adr = nc.dram_tensor("adr", [P, HC], f16, kind="ExternalInput").ap()
    F = nc.dram_tensor("F", [P, NT * 32], f16, kind="ExternalOutput").ap()

    with tile.TileContext(nc) as tc:
        with (
            tc.tile_pool(name="sbuf", bufs=1) as pool,
            tc.tile_pool(name="psum", bufs=4, space="PSUM") as psum,
        ):
            xT_sb = pool.tile([P, NP], f16)
            W_sb = pool.tile([P, HC], f16)
            asr_sb = pool.tile([P, HC], f16)
            adr_sb = pool.tile([P, HC], f16)
            Fbuf = pool.tile([P, NT, 32], f16)
            hm = pool.tile([P, NT, HC], f16)

            nc.sync.dma_start(out=xT_sb[:], in_=xT[:])
            nc.sync.dma_start(out=W_sb[:], in_=Wt[:])
            nc.sync.dma_start(out=asr_sb[:], in_=asr[:])
            nc.sync.dma_start(out=adr_sb[:], in_=adr[:])

            G1 = 4
            for _rep in range(body_reps):
                for g in range(NT // G1):
                    ph = psum.tile([P, G1 * HC], mybir.dt.float32, tag="ph")
                    for i in range(G1):
                        t = g * G1 + i
                        nc.tensor.matmul(out=ph[:, i * HC:(i + 1) * HC],
                                         lhsT=xT_sb[:, t * P:(t + 1) * P],
                                         rhs=W_sb[:], start=True, stop=True)
                    nc.scalar.copy(
                        out=Fbuf[:, g * G1:(g + 1) * G1, 0:HC],
                        in_=ph[:].rearrange("p (t f) -> p t f", f=HC))

                hview = Fbuf[:, :, 0:HC]
                for attn, sl in ((asr_sb, slice(16, 24)), (adr_sb, slice(24, 32))):
                    nc.vector.tensor_tensor(
                        out=hm[:], in0=hview,
                        in1=attn[:, None, :].broadcast_to([P, NT, HC]),
                        op=mybir.AluOpType.mult)
                    with nc.allow_low_precision(reason="2-term head dot in fp16"):
                        nc.vector.tensor_reduce(
                            out=Fbuf[:, :, sl],
                            in_=hm[:].rearrange("p t (h c) -> p t h c", c=2),
                            axis=mybir.AxisListType.X, op=mybir.AluOpType.add)

            nc.sync.dma_start(out=F.rearrange("p (t f) -> p t f", f=32), in_=Fbuf[:])
    nc.compile()
    return nc


def _build_kernel2(ptab, tilebase, S, body_reps=1):
    import concourse.bacc as bacc
    import concourse.tile as tile
    import concourse.mybir as mybir

    f16 = {"bf16": mybir.dt.bfloat16, "f16": mybir.dt.float16}[STREAM_DT]
    f32 = mybir.dt.float32
    ptab = [int(v) for v in ptab]
    tilebase = [int(v) for v in tilebase]
    nc = bacc.Bacc("TRN2", target_bir_lowering=False, debug=False,
                   enable_asserts=True, num_devices=N_CORES)
    SF = nc.dram_tensor("SF", [S * 24], f16, kind="ExternalInput").ap()
    AD = nc.dram_tensor("AD", [P, NT * HEADS], f16, kind="ExternalInput").ap()
    brep = nc.dram_tensor("brep", [P, HC], f32, kind="ExternalInput").ap()
    w0 = nc.dram_tensor("w0", [P, HC], f32, kind="ExternalInput").ap()
    w1 = nc.dram_tensor("w1", [P, HC], f32, kind="ExternalInput").ap()
    bfc = nc.dram_tensor("bfc", [P, 2], f32, kind="ExternalInput").ap()
    OUT = nc.dram_tensor("OUT", [P, NT * 2], f32, kind="ExternalOutput").ap()

    pmax = max(ptab)
    with tile.TileContext(nc) as tc:
        with tc.tile_pool(name="sbuf", bufs=1) as cpool, \
             tc.tile_pool(name="feat", bufs=2) as fpool, \
             tc.tile_pool(name="work", bufs=1) as wpool, \
             tc.tile_pool(name="pipe", bufs=2) as ppool:
            AD_sb = cpool.tile([P, NT, HEADS], f16)
            brep_sb = cpool.tile([P, HC], f32)
            w0_sb = cpool.tile([P, HC], f32)
            w1_sb = cpool.tile([P, HC], f32)
            bfc_sb = cpool.tile([P, 2], f32)
            SQ = cpool.tile([P, NT, 24], f32)
            SQl = cpool.tile([P, NT, 24], f16)
            agg = cpool.tile([P, NT, HC], f32)
            outb = cpool.tile([P, NT, 2], f32)

            nc.sync.dma_start(out=AD_sb[:], in_=AD.rearrange("p (t h) -> p t h",
                                                             h=HEADS))
            nc.sync.dma_start(out=brep_sb[:], in_=brep[:])
            nc.sync.dma_start(out=w0_sb[:], in_=w0[:])
            nc.sync.dma_start(out=w1_sb[:], in_=w1[:])
            nc.sync.dma_start(out=bfc_sb[:], in_=bfc[:])

            for _rep in range(body_reps):
                for sg in range(NT // SGT):
                    ts0 = sg * SGT
                    pt = ptab[ts0]           # equal within a super-group
                    o = tilebase[ts0] * 24   # element offset of the super-group
                    featf = fpool.tile([P, SGT * 24 * pmax], f16, tag="feat")
                    nc.sync.dma_start(
                        out=featf[:, :SGT * 24 * pt].rearrange(
                            "p (t f j) -> p t f j", t=SGT, f=24),
                        in_=SF[o:o + SGT * P * pt * 24].rearrange(
                            "(p t f j) -> p t f j", p=P, t=SGT, f=24),
                    )
                    for half in range(SGT // GRP):
                        t0 = ts0 + half * GRP
                        feat = featf[:, half * GRP * 24 * pt:
                                     (half + 1) * GRP * 24 * pt].rearrange(
                            "p (t f j) -> p t f j", t=GRP, f=24)
                        # sv/sv2 double-buffered: the ACT exp reads sv2, and
                        # with a single buffer the next group's DVE stt stalls
                        # until the scalar engine catches up
                        sv = ppool.tile([P, GRP, HEADS, pmax], f16, tag="sv")
                        sv2 = ppool.tile([P, GRP, HEADS, pmax], f16, tag="sv2")
                        rt = wpool.tile([P, GRP, 24, pmax], f16, tag="rt")
                        t1 = wpool.tile([P, GRP, 24, pmax // 2], f16, tag="t1")
                        t2 = wpool.tile([P, GRP, 24, pmax // 4], f16, tag="t2")
                        # s = a_src + a_dst  (a_dst broadcast along the slot axis)
                        nc.vector.tensor_tensor(
                            out=sv[:, :, :, :pt],
                            in0=feat[:, :, 16:24, :],
                            in1=AD_sb[:, t0:t0 + GRP, :, None].broadcast_to(
                                [P, GRP, HEADS, pt]),
                            op=mybir.AluOpType.add)
                        # v = max(0.2*s, s) on DVE (out-of-place: in-place 16-bit
                        # ops fall off the packed-uop fast path)
                        nc.vector.scalar_tensor_tensor(
                            out=sv2[:, :, :, :pt], in0=sv[:, :, :, :pt],
                            scalar=NEG_SLOPE, in1=sv[:, :, :, :pt],
                            op0=mybir.AluOpType.mult, op1=mybir.AluOpType.max)
                        nc.scalar.activation(out=rt[:, :, 16:24, :pt],
                                             in_=sv2[:, :, :, :pt],
                                             func=mybir.ActivationFunctionType.Exp)
                        # q = p * h: features are (c, h)-ordered => two dense mults
                        for c0 in (0, 8):
                            nc.vector.tensor_tensor(
                                out=rt[:, :, c0:c0 + 8, :pt],
                                in0=feat[:, :, c0:c0 + 8, :],
                                in1=rt[:, :, 16:24, :pt],
                                op=mybir.AluOpType.mult)
                        # segment sum over j: two pairwise 16-bit tree levels,
                        # then a short f32-accumulating reduce
                        h1, h2 = pt // 2, pt // 4
                        nc.vector.tensor_tensor(
                            out=t1[:, :, :, :h1], in0=rt[:, :, :, 0:h1],
                            in1=rt[:, :, :, h1:pt], op=mybir.AluOpType.add)
                        nc.vector.tensor_tensor(
                            out=t2[:, :, :, :h2], in0=t1[:, :, :, 0:h2],
                            in1=t1[:, :, :, h2:h1], op=mybir.AluOpType.add)
                        nc.vector.tensor_reduce(
                            out=SQ[:, t0:t0 + GRP, :],
                            in_=t2[:, :, :, :h2],
                            axis=mybir.AxisListType.X, op=mybir.AluOpType.add)

            rec = cpool.tile([P, NT, HEADS], f32, tag="rec")
            nc.vector.reciprocal(out=rec[:], in_=SQ[:, :, 16:24])
            nc.vector.tensor_tensor(
                out=agg[:].rearrange("p t (c h) -> p t c h", h=HEADS),
                in0=SQ[:, :, 0:16].rearrange("p t (c h) -> p t c h", h=HEADS),
                in1=rec[:, :, None, :].broadcast_to([P, NT, 2, HEADS]),
                op=mybir.AluOpType.mult)
            nc.vector.tensor_tensor(
                out=agg[:], in0=agg[:],
                in1=brep_sb[:, None, :].broadcast_to([P, NT, HC]),
                op=mybir.AluOpType.add)
            tmp = cpool.tile([P, NT, HC], f32, tag="tmp")
            outv = outb[:].rearrange("p t c -> p c t")
            for wsb, col in ((w0_sb, 0), (w1_sb, 1)):
                nc.vector.tensor_tensor(
                    out=tmp[:], in0=agg[:],
                    in1=wsb[:, None, :].broadcast_to([P, NT, HC]),
                    op=mybir.AluOpType.mult)
                nc.vector.tensor_reduce(out=outb[:, :, col], in_=tmp[:],
                                        axis=mybir.AxisListType.X,
                                        op=mybir.AluOpType.add)
            nc.vector.tensor_tensor(
                out=outb[:], in0=outb[:],
                in1=bfc_sb[:, None, :].broadcast_to([P, NT, 2]),
                op=mybir.AluOpType.add)
            nc.sync.dma_start(out=OUT.rearrange("p (t c) -> p t c", c=2),
                              in_=outb[:])
    nc.compile()
    return nc


# ------------------------------------------------------------------ runner
class _Runner:
    """Reusable jitted shard_map executor for a compiled Bacc kernel."""

    def __init__(self, nc, in_maps):
        import jax
        from jax.sharding import Mesh, PartitionSpec, NamedSharding
        from jax.experimental.shard_map import shard_map
        from concourse import bass2jax, mybir

        bass2jax.install_neuronx_cc_hook()
        partition_name = (nc.partition_id_tensor.name
                          if nc.partition_id_tensor else None)
        in_names, out_names, out_avals, zero_outs = [], [], [], []
        for alloc in nc.m.functions[0].allocations:
            if not isinstance(alloc, mybir.MemoryLocationSet):
                continue
            name = alloc.memorylocations[0].name
            if alloc.kind == "ExternalInput":
                if name != partition_name:
                    in_names.append(name)
            elif alloc.kind == "ExternalOutput":
                shape = tuple(alloc.tensor_shape)
                dtype = mybir.dt.np(alloc.dtype)
                out_names.append(name)
                out_avals.append(jax.core.ShapedArray(shape, dtype))
                zero_outs.append(np.zeros(shape, dtype))
        n_params = len(in_names)
        all_in = list(in_names) + list(out_names)
        if partition_name is not None:
            all_in.append(partition_name)

        def _body(*args):
            operands = list(args)
            if partition_name is not None:
                operands.append(bass2jax.partition_id_tensor())
            return tuple(bass2jax._bass_exec_p.bind(
                *operands, out_avals=tuple(out_avals), in_names=tuple(all_in),
                out_names=tuple(out_names), lowering_input_output_aliases=(),
                sim_require_finite=True, sim_require_nnan=True, nc=nc))

        devices = jax.devices()[:N_CORES]
        mesh = Mesh(np.asarray(devices), ("core",))
        specs = (PartitionSpec("core"),)
        self._fn = jax.jit(
            shard_map(_body, mesh=mesh,
                      in_specs=specs * (n_params + len(out_avals)),
                      out_specs=specs * len(out_avals), check_rep=False),
            keep_unused=True)
        per_core = [[np.asarray(m[name]) for name in in_names] for m in in_maps]
        concat_in = [np.concatenate([per_core[c][i] for c in range(N_CORES)], axis=0)
                     for i in range(n_params)]
        concat_zero = [np.zeros((N_CORES * z.shape[0], *z.shape[1:]), z.dtype)
                       for z in zero_outs]
        sh = NamedSharding(mesh, PartitionSpec("core"))
        self._args = [jax.device_put(a, sh) for a in concat_in + concat_zero]
        self._out_names = out_names
        self._out_avals = out_avals
        self._jax = jax

    def run(self):
        outs = self._fn(*self._args)
        return [
            {name: np.asarray(outs[i]).reshape(N_CORES, *self._out_avals[i].shape)[c]
             for i, name in enumerate(self._out_names)}
            for c in range(N_CORES)
        ]

    def time(self, iters=8, warmup=2):
        for _ in range(warmup):
            self._jax.block_until_ready(self._fn(*self._args))
        walls = []
        for _ in range(iters):
            t0 = time.perf_counter()
            self._jax.block_until_ready(self._fn(*self._args))
            walls.append(time.perf_counter() - t0)
        return min(walls)


# --------------------------------------------------------------- host glue
def _k1_maps(x, W, att_src, att_dst):
    asr = np.tile(att_src.reshape(1, HC), (P, 1)).astype(NP_DT)
    adr = np.tile(att_dst.reshape(1, HC), (P, 1)).astype(NP_DT)
    W16 = W.astype(NP_DT)
    maps1 = []
    for c in range(N_CORES):
        xT = np.zeros((P, NP), NP_DT)
        xT[:, :NODES_PER_CORE] = x[c * NODES_PER_CORE:(c + 1) * NODES_PER_CORE].T
        maps1.append({"xT": xT, "Wt": W16, "asr": asr, "adr": adr})
    return maps1


def _f_full_from_res1(res1):
    # F hbm layout [P, NT*32]: rank r=(t*P+p) at [p, t*32:(t+1)*32]
    parts = []
    for c in range(N_CORES):
        Fr = res1[c]["F"].reshape(P, NT, 32).transpose(1, 0, 2).reshape(NP, 32)
        parts.append(Fr[:NODES_PER_CORE])
    return np.concatenate(parts, axis=0)        # [N_NODES, 32]


def _k2_maps(F16_full, shards, bias_gat, W_fc, b_fc):
    S = shards["S"]
    ptab = shards["ptab"]
    tilebase = shards["tilebase"]
    brep = np.tile(bias_gat[PERM16].reshape(1, HC), (P, 1)).astype(np.float32)
    w0 = np.tile(W_fc[PERM16, 0].reshape(1, HC), (P, 1)).astype(np.float32)
    w1 = np.tile(W_fc[PERM16, 1].reshape(1, HC), (P, 1)).astype(np.float32)
    bfcr = np.tile(b_fc.reshape(1, 2), (P, 1)).astype(np.float32)
    maps2 = []
    for c in range(N_CORES):
        ssrc = shards["slot_srcs"][c]
        perm = shards["perms"][c]
        SFrow = np.zeros((S, 24), NP_DT)
        SFrow[:, 16:24] = PAD_ASRC
        real = ssrc >= 0
        SFrow[real] = F16_full[ssrc[real]][:, PERM16 + list(range(16, 24))]
        # feature-major group blocks: (p, t, f, j) flattened per group
        blocks = []
        for sg in range(NT // SGT):
            ts0 = sg * SGT
            pt = int(ptab[ts0])
            b0 = int(tilebase[ts0])
            blk = SFrow[b0:b0 + SGT * P * pt].reshape(SGT, P, pt, 24)
            blocks.append(np.ascontiguousarray(
                blk.transpose(1, 0, 3, 2)).reshape(-1))
        SF = np.concatenate(blocks)
        # AD hbm layout [P, NT*8]: rank r=(t*P+p) at [p, t*8:(t+1)*8]
        AD_rank = np.zeros((NP, HEADS), NP_DT)
        pr = perm >= 0
        AD_rank[pr] = F16_full[perm[pr], 24:32]
        AD = np.ascontiguousarray(
            AD_rank.reshape(NT, P, HEADS).transpose(1, 0, 2)).reshape(P, NT * HEADS)
        maps2.append({"SF": SF, "AD": AD, "brep": brep, "w0": w0, "w1": w1,
                      "bfc": bfcr})
    return maps2


def _out_from_res2(res2, shards):
    out = np.zeros((N_NODES, 2), np.float32)
    for c in range(N_CORES):
        perm = shards["perms"][c]
        pr = perm >= 0
        Or = res2[c]["OUT"].reshape(P, NT, 2).transpose(1, 0, 2).reshape(NP, 2)
        out[perm[pr]] = Or[pr]
    return out


# ------------------------------------------------------------------- kernel
def kernel(**inputs):
    x = np.asarray(inputs["x"], np.float32)
    edge_index = np.asarray(inputs["edge_index"])
    W = np.asarray(inputs["W"], np.float32)
    att_src = np.asarray(inputs["att_src"], np.float32)
    att_dst = np.asarray(inputs["att_dst"], np.float32)
    bias_gat = np.asarray(inputs["bias_gat"], np.float32)
    W_fc = np.asarray(inputs["W_fc"], np.float32)
    b_fc = np.asarray(inputs["b_fc"], np.float32)
    # edge_attr intentionally ignored (GATConv built without edge_dim).

    shards = _build_shards(edge_index)

    def _run_retrying(build_nc, maps, attempts=3):
        last = None
        for i in range(attempts):
            try:
                return _Runner(build_nc(), maps).run()
            except Exception as e:  # transient device desync seen on this setup
                last = e
                time.sleep(2.0)
        raise last

    res1 = _run_retrying(_build_kernel1, _k1_maps(x, W, att_src, att_dst))
    F16_full = _f_full_from_res1(res1)

    maps2 = _k2_maps(F16_full, shards, bias_gat, W_fc, b_fc)
    res2 = _run_retrying(
        lambda: _build_kernel2(shards["ptab"], shards["tilebase"], shards["S"]),
        maps2)
    return _out_from_res2(res2, shards)



# revision 19
# speedup vs baseline: 3.1188x; 3.1188x over previous
"""GAT (PyG GATConv + Linear) on 8 Trainium2 NeuronCores — v3.

Design vs the previous (v2) kernel:
  k1: ONE folded stationary matmul.  a_src/a_dst are linear in x
      (a_src = x @ (W[:,h,:] . att_src[h,:])), so F = [q(16)|a_src(8)|a_dst(8)]
      = x @ Wbig with Wbig [128, 32].  Wbig is the PE stationary; xT streams
      through in 512-node chunks, 4 chunks stacked on PSUM partitions
      (32 feats x 4 chunks = 128) so one DVE copy drains 4 chunks at 2x.
  k2: per-edge stream is [q(16) | s(8)] f16 where s = a_src[src]+a_dst[dst]
      (pre-added on host during the gather).  Each destination's edge list is
      split over 2 partition sub-rows (p^ = 2d+s); on-device per slot:
        ACT:  lrelu (Lrelu, alpha=0.2) then exp           (1-input engine)
        DVE:  p*q mult (2x) + one pairwise tree level (2x) = 14 cy/slot-lane
        PE :  kron(I64, ones(2)) stationary contracts the 2 sub-rows and
              PSUM-accumulates over residual slots (h matmuls per supergroup,
              240 cols each) -- replaces the old t2+tensor_reduce DVE tail
        ACT:  PSUM -> SBUF drain copy
      Epilogue (once): reciprocal_approx_fast for 1/Z, normalize, bias, FC.
"""
import os
import sys
import time

for _p in ("/opt/trn_rl_repo", "/root/.axon_site/_ro/trn_rl_repo"):
    if os.path.isdir(_p) and _p not in sys.path:
        sys.path.append(_p)

import numpy as np
import ml_dtypes

NP_DT = np.float16

N_NODES = 100000
N_CORES = 8
IN_F = 128
HEADS = 8
OUT_C = 2
HC = HEADS * OUT_C          # 16
NEG_SLOPE = 0.2
NODES_PER_CORE = N_NODES // N_CORES   # 12500
P = 128
D_T = 64                    # dst ranks per tile' (2 sub-rows each)
NTP = 200                   # tiles' (64 ranks each -> 12800 rank slots)
SGT = 10                    # tiles' per supergroup (shared pt)
NSG = NTP // SGT            # 20 supergroups
NP_RANKS = NTP * D_T        # 12800
PAD_S = -30000.0            # f16-safe: both exps underflow to exactly 0
# feature order inside SF rows: [q c-major (c,h) 16 | s h0..h7 8]
PERM16 = [h * OUT_C + c for c in range(OUT_C) for h in range(HEADS)]

CHUNK = 512                 # k1 node chunk (1 psum bank)
NCH = NP_RANKS // CHUNK     # 25 chunks
NGRP = (NCH + 2) // 3       # 9 groups of <=3 chunks (PE out base in {0,32,64})


# ----------------------------------------------------------------- host prep
def _build_shards(edge_index):
    src = np.asarray(edge_index[0], dtype=np.int64)
    dst = np.asarray(edge_index[1], dtype=np.int64)
    loops = np.arange(N_NODES, dtype=np.int64)
    src = np.concatenate([src, loops])
    dst = np.concatenate([dst, loops])

    deg = np.bincount(dst, minlength=N_NODES)
    order = np.argsort(-deg, kind="stable")        # global degree rank -> node
    rank_of = np.empty(N_NODES, np.int64)
    rank_of[order] = np.arange(N_NODES)
    # stripe ranks across cores: core = rank % 8, local rank rr = rank // 8
    perms = [order[c::N_CORES] for c in range(N_CORES)]   # perms[c][rr] = node

    # pt per tile' = max over its 64 ranks of ceil(deg/2); shared across cores
    degs_r = deg[order]                             # deg by global rank
    half = (degs_r + 1) // 2
    half_pad = np.zeros(NP_RANKS * N_CORES, np.int64)
    half_pad[:N_NODES] = half                       # global rank layout
    # tile' t' of core c covers global ranks {8*(64 t' + d) + c}
    hp = half_pad.reshape(NP_RANKS, N_CORES)        # [global rr slots, core]
    ptab = hp.reshape(NTP, D_T, N_CORES).max(axis=(1, 2))
    ptab = np.repeat(ptab.reshape(NSG, SGT).max(axis=1), SGT)
    ptab = np.maximum(((ptab + 1) // 2) * 2, 2)
    # slot space: per sg block [P, SGT, pt] slots; q stream 16 f16/slot,
    # s stream 8 fp8/slot, both (p, t, f, j)-ordered per sg
    sg_slots = P * SGT * ptab[::SGT]
    sgbase = np.concatenate([[0], np.cumsum(sg_slots)[:-1]])
    N_SLOTS = int(sg_slots.sum())

    core_of_dst = rank_of[dst] % N_CORES
    rr_of_dst = rank_of[dst] // N_CORES
    slot_srcs, slot_dsts = [], []
    for c in range(N_CORES):
        m = core_of_dst == c
        esrc = src[m]
        edst = dst[m]
        rr = rr_of_dst[m]
        o2 = np.argsort(rr, kind="stable")
        rr_s = rr[o2]
        esrc_s = esrc[o2]
        edst_s = edst[o2]
        _, cnt = np.unique(rr_s, return_counts=True)
        j_in = np.arange(len(rr_s)) - np.repeat(np.cumsum(cnt) - cnt, cnt)
        d_deg = np.repeat(cnt, cnt)
        # sub-row split: first ceil(deg/2) slots -> s=0, rest -> s=1
        hcount = (d_deg + 1) // 2
        s_row = (j_in >= hcount).astype(np.int64)
        j_sub = np.where(s_row == 0, j_in, j_in - hcount)
        tp = rr_s // D_T
        dd = rr_s % D_T
        phat = 2 * dd + s_row
        sg = tp // SGT
        w = tp % SGT
        pt = ptab[tp]
        # slot space: each (phat, w) row block holds pt consecutive slots
        slot_pos = sgbase[sg] + (phat * SGT + w) * pt + j_sub
        sidx = np.full(N_SLOTS, -1, np.int64)
        sdst = np.full(N_SLOTS, -1, np.int64)
        sidx[slot_pos] = esrc_s
        sdst[slot_pos] = edst_s
        slot_srcs.append(sidx)
        slot_dsts.append(sdst)

    return {"perms": perms, "ptab": ptab, "sgbase": sgbase,
            "N_SLOTS": N_SLOTS, "slot_srcs": slot_srcs, "slot_dsts": slot_dsts}


# ------------------------------------------------------------- bass kernels
def _build_kernel1(body_reps=1):
    import concourse.bacc as bacc
    import concourse.tile as tile
    import concourse.mybir as mybir

    f16 = mybir.dt.float16
    nc = bacc.Bacc("TRN2", target_bir_lowering=False, debug=False,
                   enable_asserts=True, num_devices=N_CORES)
    xT = nc.dram_tensor("xT", [P, NP_RANKS], f16, kind="ExternalInput").ap()
    WB = nc.dram_tensor("WB", [P, 32], f16, kind="ExternalInput").ap()
    F = nc.dram_tensor("F", [96, NGRP * CHUNK], f16, kind="ExternalOutput").ap()

    with tile.TileContext(nc) as tc:
        with (
            tc.tile_pool(name="sbuf", bufs=1) as pool,
            tc.tile_pool(name="xg", bufs=2) as xpool,
            tc.tile_pool(name="psum", bufs=2, space="PSUM") as psum,
        ):
            WB_sb = pool.tile([P, 32], f16)
            Fbuf = pool.tile([96, NGRP, CHUNK], f16)
            nc.sync.dma_start(out=WB_sb[:], in_=WB[:])

            for _rep in range(body_reps):
                for g in range(NGRP):
                    k_in_g = min(3, NCH - 3 * g)
                    cols = k_in_g * CHUNK
                    xg = xpool.tile([P, 3 * CHUNK], f16, tag="xg")
                    nc.sync.dma_start(
                        out=xg[:, :cols],
                        in_=xT[:, g * 3 * CHUNK:g * 3 * CHUNK + cols])
                    ph = psum.tile([96, CHUNK], mybir.dt.float32, tag="ph")
                    for k in range(k_in_g):
                        nc.tensor.matmul(
                            out=ph[32 * k:32 * (k + 1), :],
                            lhsT=WB_sb[:],
                            rhs=xg[:, k * CHUNK:(k + 1) * CHUNK],
                            start=True, stop=True)
                    nc.scalar.copy(out=Fbuf[:, g, :], in_=ph[:])

            nc.sync.dma_start(out=F.rearrange("p (g i) -> p g i", i=CHUNK),
                              in_=Fbuf[:])
    nc.compile()
    return nc


def _build_kernel2(ptab, sgbase, N_SLOTS, body_reps=1):
    import concourse.bacc as bacc
    import concourse.tile as tile
    import concourse.mybir as mybir

    f16 = mybir.dt.float16
    f32 = mybir.dt.float32
    ptab = [int(v) for v in ptab]
    sgbase = [int(v) for v in sgbase]
    nc = bacc.Bacc("TRN2", target_bir_lowering=False, debug=False,
                   enable_asserts=True, num_devices=N_CORES)
    SF = nc.dram_tensor("SF", [N_SLOTS * 24], f16, kind="ExternalInput").ap()
    brep = nc.dram_tensor("brep", [D_T, HC], f32, kind="ExternalInput").ap()
    w0 = nc.dram_tensor("w0", [D_T, HC], f32, kind="ExternalInput").ap()
    w1 = nc.dram_tensor("w1", [D_T, HC], f32, kind="ExternalInput").ap()
    bfc = nc.dram_tensor("bfc", [D_T, 2], f32, kind="ExternalInput").ap()
    KR = nc.dram_tensor("KR", [P, D_T], f16, kind="ExternalInput").ap()
    OUT = nc.dram_tensor("OUT", [D_T, NTP * 2], f32, kind="ExternalOutput").ap()

    pmax = max(ptab)
    with tile.TileContext(nc) as tc:
        with tc.tile_pool(name="const", bufs=1) as cpool, \
             tc.tile_pool(name="feat", bufs=2) as fpool, \
             tc.tile_pool(name="work", bufs=2) as wpool, \
             tc.tile_pool(name="psum", bufs=2, space="PSUM") as qpool:
            brep_sb = cpool.tile([D_T, HC], f32)
            w0_sb = cpool.tile([D_T, HC], f32)
            w1_sb = cpool.tile([D_T, HC], f32)
            bfc_sb = cpool.tile([D_T, 2], f32)
            KR_sb = cpool.tile([P, D_T], f16)
            SQ = cpool.tile([D_T, NTP, 24], f32)

            nc.sync.dma_start(out=brep_sb[:], in_=brep[:])
            nc.sync.dma_start(out=w0_sb[:], in_=w0[:])
            nc.sync.dma_start(out=w1_sb[:], in_=w1[:])
            nc.sync.dma_start(out=bfc_sb[:], in_=bfc[:])
            nc.sync.dma_start(out=KR_sb[:], in_=KR[:])

            for _rep in range(body_reps):
                for sg in range(NSG):
                    pt = ptab[sg * SGT]
                    o = sgbase[sg]
                    feat = fpool.tile([P, SGT * 24 * pmax], f16, tag="feat")
                    nc.sync.dma_start(
                        out=feat[:, :SGT * 24 * pt],
                        in_=SF[o * 24:(o + P * SGT * pt) * 24].rearrange(
                            "(p e) -> p e", p=P))
                    ft = feat[:, :SGT * 24 * pt].rearrange(
                        "p (t f j) -> p t f j", t=SGT, f=24)
                    e1 = wpool.tile([P, SGT, 8, pmax], f16, tag="e1")
                    e2 = wpool.tile([P, SGT, 8, pmax], f16, tag="e2")
                    rt = wpool.tile([P, SGT, 24, pmax], f16, tag="rt")
                    # p = exp(lrelu(s)) = max(exp(s), exp(0.2*s)) — two plain
                    # ACT Exps (exact lrelu identity; Lrelu LUT is unreliable)
                    nc.scalar.activation(
                        out=e1[:, :, :, :pt], in_=ft[:, :, 16:24, :],
                        func=mybir.ActivationFunctionType.Exp)
                    nc.scalar.activation(
                        out=e2[:, :, :, :pt], in_=ft[:, :, 16:24, :],
                        func=mybir.ActivationFunctionType.Exp,
                        scale=NEG_SLOPE)
                    nc.vector.tensor_tensor(
                        out=rt[:, :, 16:24, :pt], in0=e1[:, :, :, :pt],
                        in1=e2[:, :, :, :pt], op=mybir.AluOpType.max)
                    # q * p: c-major q => two dense 3-free-dim mults (2x)
                    for c0 in (0, 8):
                        nc.vector.tensor_tensor(
                            out=rt[:, :, c0:c0 + 8, :pt],
                            in0=ft[:, :, c0:c0 + 8, :],
                            in1=rt[:, :, 16:24, :pt],
                            op=mybir.AluOpType.mult)
                    # PE: kron(I64, ones(2)) contracts sub-row pairs and
                    # PSUM-accumulates over all pt slots
                    ps = qpool.tile([D_T, SGT * 24], f32, tag="ps")
                    for j in range(pt):
                        nc.tensor.matmul(
                            out=ps[:],
                            lhsT=KR_sb[:],
                            rhs=rt[:, :, :, j].rearrange("p t f -> p (t f)"),
                            start=(j == 0), stop=(j == pt - 1))
                    nc.scalar.copy(
                        out=SQ[:, sg * SGT:(sg + 1) * SGT, :],
                        in_=ps[:].rearrange("p (t f) -> p t f", f=24))

            # ---- epilogue (once): normalize, bias, FC
            rec = cpool.tile([D_T, NTP, HEADS], f32, tag="rec")
            agg = cpool.tile([D_T, NTP, HC], f32, tag="agg")
            outb = cpool.tile([D_T, NTP, 2], f32, tag="outb")
            nc.vector.reciprocal_approx_fast(out=rec[:], in_=SQ[:, :, 16:24])
            nc.vector.tensor_tensor(
                out=agg[:].rearrange("p t (c h) -> p t c h", c=OUT_C),
                in0=SQ[:, :, 0:16].rearrange("p t (c h) -> p t c h", c=OUT_C),
                in1=rec[:, :, None, :].broadcast_to([D_T, NTP, OUT_C, HEADS]),
                op=mybir.AluOpType.mult)
            nc.vector.tensor_tensor(
                out=agg[:], in0=agg[:],
                in1=brep_sb[:, None, :].broadcast_to([D_T, NTP, HC]),
                op=mybir.AluOpType.add)
            tmp = cpool.tile([D_T, NTP, HC], f32, tag="tmp")
            for wsb, col in ((w0_sb, 0), (w1_sb, 1)):
                nc.vector.tensor_tensor(
                    out=tmp[:], in0=agg[:],
                    in1=wsb[:, None, :].broadcast_to([D_T, NTP, HC]),
                    op=mybir.AluOpType.mult)
                nc.vector.tensor_reduce(out=outb[:, :, col], in_=tmp[:],
                                        axis=mybir.AxisListType.X,
                                        op=mybir.AluOpType.add)
            nc.vector.tensor_tensor(
                out=outb[:], in0=outb[:],
                in1=bfc_sb[:, None, :].broadcast_to([D_T, NTP, 2]),
                op=mybir.AluOpType.add)
            nc.sync.dma_start(out=OUT.rearrange("p (t c) -> p t c", c=2),
                              in_=outb[:])
    nc.compile()
    return nc


# ------------------------------------------------------------------ runner
class _Runner:
    """Reusable jitted shard_map executor for a compiled Bacc kernel."""

    def __init__(self, nc, in_maps):
        import jax
        from jax.sharding import Mesh, PartitionSpec, NamedSharding
        from jax.experimental.shard_map import shard_map
        from concourse import bass2jax, mybir

        bass2jax.install_neuronx_cc_hook()
        partition_name = (nc.partition_id_tensor.name
                          if nc.partition_id_tensor else None)
        in_names, out_names, out_avals, zero_outs = [], [], [], []
        for alloc in nc.m.functions[0].allocations:
            if not isinstance(alloc, mybir.MemoryLocationSet):
                continue
            name = alloc.memorylocations[0].name
            if alloc.kind == "ExternalInput":
                if name != partition_name:
                    in_names.append(name)
            elif alloc.kind == "ExternalOutput":
                shape = tuple(alloc.tensor_shape)
                dtype = mybir.dt.np(alloc.dtype)
                out_names.append(name)
                out_avals.append(jax.core.ShapedArray(shape, dtype))
                zero_outs.append(np.zeros(shape, dtype))
        n_params = len(in_names)
        all_in = list(in_names) + list(out_names)
        if partition_name is not None:
            all_in.append(partition_name)

        def _body(*args):
            operands = list(args)
            if partition_name is not None:
                operands.append(bass2jax.partition_id_tensor())
            return tuple(bass2jax._bass_exec_p.bind(
                *operands, out_avals=tuple(out_avals), in_names=tuple(all_in),
                out_names=tuple(out_names), lowering_input_output_aliases=(),
                sim_require_finite=False, sim_require_nnan=False, nc=nc))

        devices = jax.devices()[:N_CORES]
        mesh = Mesh(np.asarray(devices), ("core",))
        specs = (PartitionSpec("core"),)
        self._fn = jax.jit(
            shard_map(_body, mesh=mesh,
                      in_specs=specs * (n_params + len(out_avals)),
                      out_specs=specs * len(out_avals), check_rep=False),
            keep_unused=True)
        per_core = [[np.asarray(m[name]) for name in in_names] for m in in_maps]
        concat_in = [np.concatenate([per_core[c][i] for c in range(N_CORES)], axis=0)
                     for i in range(n_params)]
        concat_zero = [np.zeros((N_CORES * z.shape[0], *z.shape[1:]), z.dtype)
                       for z in zero_outs]
        sh = NamedSharding(mesh, PartitionSpec("core"))
        self._args = [jax.device_put(a, sh) for a in concat_in + concat_zero]
        self._out_names = out_names
        self._out_avals = out_avals
        self._jax = jax

    def run(self):
        outs = self._fn(*self._args)
        return [
            {name: np.asarray(outs[i]).reshape(N_CORES, *self._out_avals[i].shape)[c]
             for i, name in enumerate(self._out_names)}
            for c in range(N_CORES)
        ]

    def time(self, iters=8, warmup=2):
        for _ in range(warmup):
            self._jax.block_until_ready(self._fn(*self._args))
        walls = []
        for _ in range(iters):
            t0 = time.perf_counter()
            self._jax.block_until_ready(self._fn(*self._args))
            walls.append(time.perf_counter() - t0)
        return min(walls)


# --------------------------------------------------------------- host glue
def _k1_maps(x, W, att_src, att_dst, shards):
    # Wbig: [q cols (c-major PERM16) | w_src | w_dst]
    Wq = W[:, PERM16]                                    # [128, 16]
    w_src = np.stack([W[:, 2 * h] * att_src[h, 0] + W[:, 2 * h + 1] * att_src[h, 1]
                      for h in range(HEADS)], axis=1)    # [128, 8]
    w_dst = np.stack([W[:, 2 * h] * att_dst[h, 0] + W[:, 2 * h + 1] * att_dst[h, 1]
                      for h in range(HEADS)], axis=1)
    WB = np.concatenate([Wq, w_src, w_dst], axis=1).astype(NP_DT)  # [128, 32]
    maps1 = []
    for c in range(N_CORES):
        perm = shards["perms"][c]
        xT = np.zeros((P, NP_RANKS), NP_DT)
        xT[:, :len(perm)] = x[perm].T
        maps1.append({"xT": xT, "WB": WB})
    return maps1


def _f_full_from_res1(res1, shards):
    F16_full = np.zeros((N_NODES, 32), NP_DT)
    for c in range(N_CORES):
        Fc = res1[c]["F"].reshape(96, NGRP, CHUNK)
        # partition q = 32*k + f holds chunk 3g+k; node rank rr = (3g+k)*512+i
        Fr = Fc.reshape(3, 32, NGRP, CHUNK).transpose(2, 0, 3, 1).reshape(-1, 32)
        perm = shards["perms"][c]
        F16_full[perm] = Fr[:len(perm)]
    return F16_full


def _k2_maps(F16_full, shards, bias_gat, W_fc, b_fc):
    N_SLOTS = shards["N_SLOTS"]
    ptab = shards["ptab"]
    sgbase = shards["sgbase"]
    brep = np.tile(bias_gat[PERM16].reshape(1, HC), (D_T, 1)).astype(np.float32)
    w0 = np.tile(W_fc[PERM16, 0].reshape(1, HC), (D_T, 1)).astype(np.float32)
    w1 = np.tile(W_fc[PERM16, 1].reshape(1, HC), (D_T, 1)).astype(np.float32)
    bfcr = np.tile(b_fc.reshape(1, 2), (D_T, 1)).astype(np.float32)
    KR = np.zeros((P, D_T), NP_DT)
    KR[np.arange(P), np.arange(P) // 2] = 1.0
    q16 = F16_full[:, 0:16]
    asrc = F16_full[:, 16:24].astype(np.float32)
    adst = F16_full[:, 24:32].astype(np.float32)
    maps2 = []
    for c in range(N_CORES):
        ssrc = shards["slot_srcs"][c]
        sdst = shards["slot_dsts"][c]
        SFrow = np.zeros((N_SLOTS, 24), NP_DT)
        SFrow[:, 16:24] = PAD_S
        real = ssrc >= 0
        SFrow[real, 0:16] = q16[ssrc[real]]
        SFrow[real, 16:24] = (asrc[ssrc[real]] + adst[sdst[real]]).astype(NP_DT)
        # expand each (row-block, j) slot run to (f, j)-major per sg
        SF = np.zeros(N_SLOTS * 24, NP_DT)
        for sg in range(NSG):
            pt = int(ptab[sg * SGT])
            o = int(sgbase[sg])
            nrows = P * SGT
            blk = SFrow[o:o + nrows * pt].reshape(nrows, pt, 24)
            SF[o * 24:(o + nrows * pt) * 24] = (
                blk.transpose(0, 2, 1).reshape(-1))
        maps2.append({"SF": SF, "brep": brep, "w0": w0, "w1": w1,
                      "bfc": bfcr, "KR": KR})
    return maps2


def _out_from_res2(res2, shards):
    out = np.zeros((N_NODES, 2), np.float32)
    for c in range(N_CORES):
        perm = shards["perms"][c]
        Or = res2[c]["OUT"].reshape(D_T, NTP, 2)
        # rank rr = t'*64 + d  ->  row d, tile t'
        Orank = Or.transpose(1, 0, 2).reshape(NP_RANKS, 2)
        out[perm] = Orank[:len(perm)]
    return out


# ------------------------------------------------------------------- kernel
def kernel(**inputs):
    x = np.asarray(inputs["x"], np.float32)
    edge_index = np.asarray(inputs["edge_index"])
    W = np.asarray(inputs["W"], np.float32)
    att_src = np.asarray(inputs["att_src"], np.float32)
    att_dst = np.asarray(inputs["att_dst"], np.float32)
    bias_gat = np.asarray(inputs["bias_gat"], np.float32)
    W_fc = np.asarray(inputs["W_fc"], np.float32)
    b_fc = np.asarray(inputs["b_fc"], np.float32)
    # edge_attr intentionally ignored (GATConv built without edge_dim).

    shards = _build_shards(edge_index)

    def _run_retrying(build_nc, maps, attempts=3):
        last = None
        for i in range(attempts):
            try:
                return _Runner(build_nc(), maps).run()
            except Exception as e:  # transient device desync seen on this setup
                last = e
                time.sleep(2.0)
        raise last

    res1 = _run_retrying(_build_kernel1, _k1_maps(x, W, att_src, att_dst, shards))
    F16_full = _f_full_from_res1(res1, shards)

    maps2 = _k2_maps(F16_full, shards, bias_gat, W_fc, b_fc)
    res2 = _run_retrying(
        lambda: _build_kernel2(shards["ptab"], shards["sgbase"],
                               shards["N_SLOTS"]),
        maps2)
    return _out_from_res2(res2, shards)


# revision 21
# speedup vs baseline: 5.5543x; 1.7809x over previous
"""GAT (PyG GATConv + Linear) on 8 Trainium2 NeuronCores — v3.

Design vs the previous (v2) kernel:
  k1: ONE folded stationary matmul.  a_src/a_dst are linear in x
      (a_src = x @ (W[:,h,:] . att_src[h,:])), so F = [q(16)|a_src(8)|a_dst(8)]
      = x @ Wbig with Wbig [128, 32].  Wbig is the PE stationary; xT streams
      through in 512-node chunks, 4 chunks stacked on PSUM partitions
      (32 feats x 4 chunks = 128) so one DVE copy drains 4 chunks at 2x.
  k2: per-edge stream is [q(16) | s(8)] f16 where s = a_src[src]+a_dst[dst]
      (pre-added on host during the gather).  Each destination's edge list is
      split over 2 partition sub-rows (p^ = 2d+s); on-device per slot:
        ACT:  lrelu (Lrelu, alpha=0.2) then exp           (1-input engine)
        DVE:  p*q mult (2x) + one pairwise tree level (2x) = 14 cy/slot-lane
        PE :  kron(I64, ones(2)) stationary contracts the 2 sub-rows and
              PSUM-accumulates over residual slots (h matmuls per supergroup,
              240 cols each) -- replaces the old t2+tensor_reduce DVE tail
        ACT:  PSUM -> SBUF drain copy
      Epilogue (once): reciprocal_approx_fast for 1/Z, normalize, bias, FC.
"""
import os
import sys
import time

for _p in ("/opt/trn_rl_repo", "/root/.axon_site/_ro/trn_rl_repo"):
    if os.path.isdir(_p) and _p not in sys.path:
        sys.path.append(_p)

import numpy as np
import ml_dtypes

NP_DT = np.float16

N_NODES = 100000
N_CORES = 8
IN_F = 128
HEADS = 8
OUT_C = 2
HC = HEADS * OUT_C          # 16
NEG_SLOPE = 0.2
NODES_PER_CORE = N_NODES // N_CORES   # 12500
P = 128
D_T = 64                    # dst ranks per tile' (2 sub-rows each)
NTP = 200                   # tiles' (64 ranks each -> 12800 rank slots)
SGT = 10                    # tiles' per supergroup (shared pt)
NSG = NTP // SGT            # 20 supergroups
NP_RANKS = NTP * D_T        # 12800
PAD_S = -30000.0            # f16-safe: both exps underflow to exactly 0
# feature order inside SF rows: [q c-major (c,h) 16 | s h0..h7 8]
PERM16 = [h * OUT_C + c for c in range(OUT_C) for h in range(HEADS)]

CHUNK = 512                 # k1 node chunk (1 psum bank)
NCH = NP_RANKS // CHUNK     # 25 chunks
NGRP = (NCH + 2) // 3       # 9 groups of <=3 chunks (PE out base in {0,32,64})


# ----------------------------------------------------------------- host prep
def _build_shards(edge_index):
    src = np.asarray(edge_index[0], dtype=np.int64)
    dst = np.asarray(edge_index[1], dtype=np.int64)
    loops = np.arange(N_NODES, dtype=np.int64)
    src = np.concatenate([src, loops])
    dst = np.concatenate([dst, loops])

    deg = np.bincount(dst, minlength=N_NODES)
    order = np.argsort(-deg, kind="stable")        # global degree rank -> node
    rank_of = np.empty(N_NODES, np.int64)
    rank_of[order] = np.arange(N_NODES)
    # stripe ranks across cores: core = rank % 8, local rank rr = rank // 8
    perms = [order[c::N_CORES] for c in range(N_CORES)]   # perms[c][rr] = node

    # pt per tile' = max over its 64 ranks of ceil(deg/2); shared across cores
    degs_r = deg[order]                             # deg by global rank
    half = (degs_r + 1) // 2
    half_pad = np.zeros(NP_RANKS * N_CORES, np.int64)
    half_pad[:N_NODES] = half                       # global rank layout
    # tile' t' of core c covers global ranks {8*(64 t' + d) + c}
    hp = half_pad.reshape(NP_RANKS, N_CORES)        # [global rr slots, core]
    ptab = hp.reshape(NTP, D_T, N_CORES).max(axis=(1, 2))
    ptab = np.repeat(ptab.reshape(NSG, SGT).max(axis=1), SGT)
    ptab = np.maximum(((ptab + 1) // 2) * 2, 2)
    # slot space: per sg block [P, SGT, pt] slots; q stream 16 f16/slot,
    # s stream 8 fp8/slot, both (p, t, f, j)-ordered per sg
    sg_slots = P * SGT * ptab[::SGT]
    sgbase = np.concatenate([[0], np.cumsum(sg_slots)[:-1]])
    N_SLOTS = int(sg_slots.sum())

    core_of_dst = rank_of[dst] % N_CORES
    rr_of_dst = rank_of[dst] // N_CORES
    slot_srcs, slot_dsts = [], []
    for c in range(N_CORES):
        m = core_of_dst == c
        esrc = src[m]
        edst = dst[m]
        rr = rr_of_dst[m]
        o2 = np.argsort(rr, kind="stable")
        rr_s = rr[o2]
        esrc_s = esrc[o2]
        edst_s = edst[o2]
        _, cnt = np.unique(rr_s, return_counts=True)
        j_in = np.arange(len(rr_s)) - np.repeat(np.cumsum(cnt) - cnt, cnt)
        d_deg = np.repeat(cnt, cnt)
        # sub-row split: first ceil(deg/2) slots -> s=0, rest -> s=1
        hcount = (d_deg + 1) // 2
        s_row = (j_in >= hcount).astype(np.int64)
        j_sub = np.where(s_row == 0, j_in, j_in - hcount)
        tp = rr_s // D_T
        dd = rr_s % D_T
        phat = 2 * dd + s_row
        sg = tp // SGT
        w = tp % SGT
        pt = ptab[tp]
        # slot space: each (phat, w) row block holds pt consecutive slots
        slot_pos = sgbase[sg] + (phat * SGT + w) * pt + j_sub
        sidx = np.full(N_SLOTS, -1, np.int64)
        sdst = np.full(N_SLOTS, -1, np.int64)
        sidx[slot_pos] = esrc_s
        sdst[slot_pos] = edst_s
        slot_srcs.append(sidx)
        slot_dsts.append(sdst)

    return {"perms": perms, "ptab": ptab, "sgbase": sgbase,
            "N_SLOTS": N_SLOTS, "slot_srcs": slot_srcs, "slot_dsts": slot_dsts}


# ------------------------------------------------------------- bass kernels
def _build_kernel1(body_reps=1):
    import concourse.bacc as bacc
    import concourse.tile as tile
    import concourse.mybir as mybir

    f16 = mybir.dt.float16
    nc = bacc.Bacc("TRN2", target_bir_lowering=False, debug=False,
                   enable_asserts=True, num_devices=N_CORES)
    xT = nc.dram_tensor("xT", [P, NP_RANKS], f16, kind="ExternalInput").ap()
    WB = nc.dram_tensor("WB", [P, 32], f16, kind="ExternalInput").ap()
    F = nc.dram_tensor("F", [96, NGRP * CHUNK], f16, kind="ExternalOutput").ap()

    with tile.TileContext(nc) as tc:
        with (
            tc.tile_pool(name="sbuf", bufs=1) as pool,
            tc.tile_pool(name="xg", bufs=2) as xpool,
            tc.tile_pool(name="psum", bufs=4, space="PSUM") as psum,
        ):
            WB_sb = pool.tile([P, 32], f16)
            Fbuf = pool.tile([96, NGRP, CHUNK], f16)
            nc.sync.dma_start(out=WB_sb[:], in_=WB[:])

            HGRP = 3                       # NGRP groups per DMA half-buffer
            for _rep in range(body_reps):
                for half in range((NGRP + HGRP - 1) // HGRP):
                    g0 = half * HGRP
                    g1 = min(g0 + HGRP, NGRP)
                    cols = min(3 * CHUNK * g1, NP_RANKS) - 3 * CHUNK * g0
                    xh = xpool.tile([P, HGRP * 3 * CHUNK], f16, tag="xh")
                    nc.sync.dma_start(
                        out=xh[:, :cols],
                        in_=xT[:, g0 * 3 * CHUNK:g0 * 3 * CHUNK + cols])
                    for g in range(g0, g1):
                        k_in_g = min(3, NCH - 3 * g)
                        ph = psum.tile([96, CHUNK], mybir.dt.float32, tag="ph")
                        for k in range(k_in_g):
                            c0 = (g - g0) * 3 * CHUNK + k * CHUNK
                            nc.tensor.matmul(
                                out=ph[32 * k:32 * (k + 1), :],
                                lhsT=WB_sb[:],
                                rhs=xh[:, c0:c0 + CHUNK],
                                start=True, stop=True)
                        nc.scalar.copy(out=Fbuf[:, g, :], in_=ph[:])

            nc.sync.dma_start(out=F.rearrange("p (g i) -> p g i", i=CHUNK),
                              in_=Fbuf[:])
    nc.compile()
    return nc


def _build_kernel2(ptab, sgbase, N_SLOTS, body_reps=1):
    import concourse.bacc as bacc
    import concourse.tile as tile
    import concourse.mybir as mybir

    f16 = mybir.dt.float16
    f32 = mybir.dt.float32
    ptab = [int(v) for v in ptab]
    sgbase = [int(v) for v in sgbase]
    nc = bacc.Bacc("TRN2", target_bir_lowering=False, debug=False,
                   enable_asserts=True, num_devices=N_CORES)
    SF = nc.dram_tensor("SF", [N_SLOTS * 24], f16, kind="ExternalInput").ap()
    brep = nc.dram_tensor("brep", [D_T, HC], f32, kind="ExternalInput").ap()
    w0 = nc.dram_tensor("w0", [D_T, HC], f32, kind="ExternalInput").ap()
    w1 = nc.dram_tensor("w1", [D_T, HC], f32, kind="ExternalInput").ap()
    bfc = nc.dram_tensor("bfc", [D_T, 2], f32, kind="ExternalInput").ap()
    KR = nc.dram_tensor("KR", [P, D_T], f16, kind="ExternalInput").ap()
    OUT = nc.dram_tensor("OUT", [D_T, NTP * 2], f32, kind="ExternalOutput").ap()

    pmax = max(ptab)
    with tile.TileContext(nc) as tc:
        with tc.tile_pool(name="const", bufs=1) as cpool, \
             tc.tile_pool(name="feat", bufs=2) as fpool, \
             tc.tile_pool(name="work", bufs=2) as wpool, \
             tc.tile_pool(name="psum", bufs=2, space="PSUM") as qpool:
            brep_sb = cpool.tile([D_T, HC], f32)
            w0_sb = cpool.tile([D_T, HC], f32)
            w1_sb = cpool.tile([D_T, HC], f32)
            bfc_sb = cpool.tile([D_T, 2], f32)
            KR_sb = cpool.tile([P, D_T], f16)
            SQ = cpool.tile([D_T, NTP, 24], f32)

            nc.sync.dma_start(out=brep_sb[:], in_=brep[:])
            nc.sync.dma_start(out=w0_sb[:], in_=w0[:])
            nc.sync.dma_start(out=w1_sb[:], in_=w1[:])
            nc.sync.dma_start(out=bfc_sb[:], in_=bfc[:])
            nc.sync.dma_start(out=KR_sb[:], in_=KR[:])

            for _rep in range(body_reps):
                for sg in range(NSG):
                    pt = ptab[sg * SGT]
                    o = sgbase[sg]
                    feat = fpool.tile([P, SGT * 24 * pmax], f16, tag="feat")
                    nc.sync.dma_start(
                        out=feat[:, :SGT * 24 * pt],
                        in_=SF[o * 24:(o + P * SGT * pt) * 24].rearrange(
                            "(p e) -> p e", p=P))
                    ft = feat[:, :SGT * 24 * pt].rearrange(
                        "p (t f j) -> p t f j", t=SGT, f=24)
                    e1 = wpool.tile([P, SGT, 8, pmax], f16, tag="e1")
                    e2 = wpool.tile([P, SGT, 8, pmax], f16, tag="e2")
                    rt = wpool.tile([P, SGT, 24, pmax], f16, tag="rt")
                    # p = exp(lrelu(s)) = max(exp(s), exp(0.2*s)) — two plain
                    # ACT Exps (exact lrelu identity; Lrelu LUT is unreliable)
                    nc.scalar.activation(
                        out=e1[:, :, :, :pt], in_=ft[:, :, 16:24, :],
                        func=mybir.ActivationFunctionType.Exp)
                    nc.scalar.activation(
                        out=e2[:, :, :, :pt], in_=ft[:, :, 16:24, :],
                        func=mybir.ActivationFunctionType.Exp,
                        scale=NEG_SLOPE)
                    nc.vector.tensor_tensor(
                        out=rt[:, :, 16:24, :pt], in0=e1[:, :, :, :pt],
                        in1=e2[:, :, :, :pt], op=mybir.AluOpType.max)
                    # q * p: c-major q => two dense 3-free-dim mults (2x)
                    for c0 in (0, 8):
                        nc.vector.tensor_tensor(
                            out=rt[:, :, c0:c0 + 8, :pt],
                            in0=ft[:, :, c0:c0 + 8, :],
                            in1=rt[:, :, 16:24, :pt],
                            op=mybir.AluOpType.mult)
                    # PE: kron(I64, ones(2)) contracts sub-row pairs and
                    # PSUM-accumulates over all pt slots
                    ps = qpool.tile([D_T, SGT * 24], f32, tag="ps")
                    for j in range(pt):
                        nc.tensor.matmul(
                            out=ps[:],
                            lhsT=KR_sb[:],
                            rhs=rt[:, :, :, j].rearrange("p t f -> p (t f)"),
                            start=(j == 0), stop=(j == pt - 1))
                    nc.scalar.copy(
                        out=SQ[:, sg * SGT:(sg + 1) * SGT, :],
                        in_=ps[:].rearrange("p (t f) -> p t f", f=24))

            # ---- epilogue (once): normalize, bias, FC
            rec = cpool.tile([D_T, NTP, HEADS], f32, tag="rec")
            agg = cpool.tile([D_T, NTP, HC], f32, tag="agg")
            outb = cpool.tile([D_T, NTP, 2], f32, tag="outb")
            nc.vector.reciprocal_approx_fast(out=rec[:], in_=SQ[:, :, 16:24])
            nc.vector.tensor_tensor(
                out=agg[:].rearrange("p t (c h) -> p t c h", c=OUT_C),
                in0=SQ[:, :, 0:16].rearrange("p t (c h) -> p t c h", c=OUT_C),
                in1=rec[:, :, None, :].broadcast_to([D_T, NTP, OUT_C, HEADS]),
                op=mybir.AluOpType.mult)
            nc.vector.tensor_tensor(
                out=agg[:], in0=agg[:],
                in1=brep_sb[:, None, :].broadcast_to([D_T, NTP, HC]),
                op=mybir.AluOpType.add)
            tmp = cpool.tile([D_T, NTP, HC], f32, tag="tmp")
            for wsb, col in ((w0_sb, 0), (w1_sb, 1)):
                nc.vector.tensor_tensor(
                    out=tmp[:], in0=agg[:],
                    in1=wsb[:, None, :].broadcast_to([D_T, NTP, HC]),
                    op=mybir.AluOpType.mult)
                nc.vector.tensor_reduce(out=outb[:, :, col], in_=tmp[:],
                                        axis=mybir.AxisListType.X,
                                        op=mybir.AluOpType.add)
            nc.vector.tensor_tensor(
                out=outb[:], in0=outb[:],
                in1=bfc_sb[:, None, :].broadcast_to([D_T, NTP, 2]),
                op=mybir.AluOpType.add)
            nc.sync.dma_start(out=OUT.rearrange("p (t c) -> p t c", c=2),
                              in_=outb[:])
    nc.compile()
    return nc


# ------------------------------------------------------------------ runner
class _Runner:
    """Reusable jitted shard_map executor for a compiled Bacc kernel."""

    def __init__(self, nc, in_maps):
        import jax
        from jax.sharding import Mesh, PartitionSpec, NamedSharding
        from jax.experimental.shard_map import shard_map
        from concourse import bass2jax, mybir

        bass2jax.install_neuronx_cc_hook()
        partition_name = (nc.partition_id_tensor.name
                          if nc.partition_id_tensor else None)
        in_names, out_names, out_avals, zero_outs = [], [], [], []
        for alloc in nc.m.functions[0].allocations:
            if not isinstance(alloc, mybir.MemoryLocationSet):
                continue
            name = alloc.memorylocations[0].name
            if alloc.kind == "ExternalInput":
                if name != partition_name:
                    in_names.append(name)
            elif alloc.kind == "ExternalOutput":
                shape = tuple(alloc.tensor_shape)
                dtype = mybir.dt.np(alloc.dtype)
                out_names.append(name)
                out_avals.append(jax.core.ShapedArray(shape, dtype))
                zero_outs.append(np.zeros(shape, dtype))
        n_params = len(in_names)
        all_in = list(in_names) + list(out_names)
        if partition_name is not None:
            all_in.append(partition_name)

        def _body(*args):
            operands = list(args)
            if partition_name is not None:
                operands.append(bass2jax.partition_id_tensor())
            return tuple(bass2jax._bass_exec_p.bind(
                *operands, out_avals=tuple(out_avals), in_names=tuple(all_in),
                out_names=tuple(out_names), lowering_input_output_aliases=(),
                sim_require_finite=False, sim_require_nnan=False, nc=nc))

        devices = jax.devices()[:N_CORES]
        mesh = Mesh(np.asarray(devices), ("core",))
        specs = (PartitionSpec("core"),)
        self._fn = jax.jit(
            shard_map(_body, mesh=mesh,
                      in_specs=specs * (n_params + len(out_avals)),
                      out_specs=specs * len(out_avals), check_rep=False),
            keep_unused=True)
        per_core = [[np.asarray(m[name]) for name in in_names] for m in in_maps]
        concat_in = [np.concatenate([per_core[c][i] for c in range(N_CORES)], axis=0)
                     for i in range(n_params)]
        concat_zero = [np.zeros((N_CORES * z.shape[0], *z.shape[1:]), z.dtype)
                       for z in zero_outs]
        sh = NamedSharding(mesh, PartitionSpec("core"))
        self._args = [jax.device_put(a, sh) for a in concat_in + concat_zero]
        self._out_names = out_names
        self._out_avals = out_avals
        self._jax = jax

    def run(self):
        outs = self._fn(*self._args)
        return [
            {name: np.asarray(outs[i]).reshape(N_CORES, *self._out_avals[i].shape)[c]
             for i, name in enumerate(self._out_names)}
            for c in range(N_CORES)
        ]

    def time(self, iters=8, warmup=2):
        for _ in range(warmup):
            self._jax.block_until_ready(self._fn(*self._args))
        walls = []
        for _ in range(iters):
            t0 = time.perf_counter()
            self._jax.block_until_ready(self._fn(*self._args))
            walls.append(time.perf_counter() - t0)
        return min(walls)


# --------------------------------------------------------------- host glue
def _k1_maps(x, W, att_src, att_dst, shards):
    # Wbig: [q cols (c-major PERM16) | w_src | w_dst]
    Wq = W[:, PERM16]                                    # [128, 16]
    w_src = np.stack([W[:, 2 * h] * att_src[h, 0] + W[:, 2 * h + 1] * att_src[h, 1]
                      for h in range(HEADS)], axis=1)    # [128, 8]
    w_dst = np.stack([W[:, 2 * h] * att_dst[h, 0] + W[:, 2 * h + 1] * att_dst[h, 1]
                      for h in range(HEADS)], axis=1)
    WB = np.concatenate([Wq, w_src, w_dst], axis=1).astype(NP_DT)  # [128, 32]
    maps1 = []
    for c in range(N_CORES):
        perm = shards["perms"][c]
        xT = np.zeros((P, NP_RANKS), NP_DT)
        xT[:, :len(perm)] = x[perm].T
        maps1.append({"xT": xT, "WB": WB})
    return maps1


def _f_full_from_res1(res1, shards):
    F16_full = np.zeros((N_NODES, 32), NP_DT)
    for c in range(N_CORES):
        Fc = res1[c]["F"].reshape(96, NGRP, CHUNK)
        # partition q = 32*k + f holds chunk 3g+k; node rank rr = (3g+k)*512+i
        Fr = Fc.reshape(3, 32, NGRP, CHUNK).transpose(2, 0, 3, 1).reshape(-1, 32)
        perm = shards["perms"][c]
        F16_full[perm] = Fr[:len(perm)]
    return F16_full


def _k2_maps(F16_full, shards, bias_gat, W_fc, b_fc):
    N_SLOTS = shards["N_SLOTS"]
    ptab = shards["ptab"]
    sgbase = shards["sgbase"]
    brep = np.tile(bias_gat[PERM16].reshape(1, HC), (D_T, 1)).astype(np.float32)
    w0 = np.tile(W_fc[PERM16, 0].reshape(1, HC), (D_T, 1)).astype(np.float32)
    w1 = np.tile(W_fc[PERM16, 1].reshape(1, HC), (D_T, 1)).astype(np.float32)
    bfcr = np.tile(b_fc.reshape(1, 2), (D_T, 1)).astype(np.float32)
    KR = np.zeros((P, D_T), NP_DT)
    KR[np.arange(P), np.arange(P) // 2] = 1.0
    q16 = F16_full[:, 0:16]
    asrc = F16_full[:, 16:24].astype(np.float32)
    adst = F16_full[:, 24:32].astype(np.float32)
    maps2 = []
    for c in range(N_CORES):
        ssrc = shards["slot_srcs"][c]
        sdst = shards["slot_dsts"][c]
        SFrow = np.zeros((N_SLOTS, 24), NP_DT)
        SFrow[:, 16:24] = PAD_S
        real = ssrc >= 0
        SFrow[real, 0:16] = q16[ssrc[real]]
        SFrow[real, 16:24] = (asrc[ssrc[real]] + adst[sdst[real]]).astype(NP_DT)
        # expand each (row-block, j) slot run to (f, j)-major per sg
        SF = np.zeros(N_SLOTS * 24, NP_DT)
        for sg in range(NSG):
            pt = int(ptab[sg * SGT])
            o = int(sgbase[sg])
            nrows = P * SGT
            blk = SFrow[o:o + nrows * pt].reshape(nrows, pt, 24)
            SF[o * 24:(o + nrows * pt) * 24] = (
                blk.transpose(0, 2, 1).reshape(-1))
        maps2.append({"SF": SF, "brep": brep, "w0": w0, "w1": w1,
                      "bfc": bfcr, "KR": KR})
    return maps2


def _out_from_res2(res2, shards):
    out = np.zeros((N_NODES, 2), np.float32)
    for c in range(N_CORES):
        perm = shards["perms"][c]
        Or = res2[c]["OUT"].reshape(D_T, NTP, 2)
        # rank rr = t'*64 + d  ->  row d, tile t'
        Orank = Or.transpose(1, 0, 2).reshape(NP_RANKS, 2)
        out[perm] = Orank[:len(perm)]
    return out


# ------------------------------------------------------------------- kernel
def kernel(**inputs):
    x = np.asarray(inputs["x"], np.float32)
    edge_index = np.asarray(inputs["edge_index"])
    W = np.asarray(inputs["W"], np.float32)
    att_src = np.asarray(inputs["att_src"], np.float32)
    att_dst = np.asarray(inputs["att_dst"], np.float32)
    bias_gat = np.asarray(inputs["bias_gat"], np.float32)
    W_fc = np.asarray(inputs["W_fc"], np.float32)
    b_fc = np.asarray(inputs["b_fc"], np.float32)
    # edge_attr intentionally ignored (GATConv built without edge_dim).

    shards = _build_shards(edge_index)

    def _run_retrying(build_nc, maps, attempts=3):
        last = None
        for i in range(attempts):
            try:
                return _Runner(build_nc(), maps).run()
            except Exception as e:  # transient device desync seen on this setup
                last = e
                time.sleep(2.0)
        raise last

    res1 = _run_retrying(_build_kernel1, _k1_maps(x, W, att_src, att_dst, shards))
    F16_full = _f_full_from_res1(res1, shards)

    maps2 = _k2_maps(F16_full, shards, bias_gat, W_fc, b_fc)
    res2 = _run_retrying(
        lambda: _build_kernel2(shards["ptab"], shards["sgbase"],
                               shards["N_SLOTS"]),
        maps2)
    return _out_from_res2(res2, shards)
